# revision 1
# baseline (speedup 1.0000x reference)
"""GatedGCN LocalEncoder kernel for 8x Trainium2 NeuronCores (Bass/Tile).

Strategy: destination-sorted edge sharding. Nodes are relabeled into
degree-balanced 128-node blocks (100352 padded slots, 784 blocks, 98 per
core). All edges with dst in a block form one contiguous, padded run, so
segment_sum becomes a one-hot matmul accumulated in PSUM per block with no
cross-core communication. Random-access per edge is limited to two
indirect-DMA gathers from per-core bf16 tables (AU by src, Vh by dst, with
the Vh gather DMA-accumulated onto the Ah region so one identity-matmul
adds both into the gate PSUM).
"""

import os
import sys
from contextlib import ExitStack

for _p in ("/opt/trn_rl_repo", os.path.expanduser("~/.axon_site/_ro/trn_rl_repo")):
    if os.path.isdir(_p) and _p not in sys.path:
        sys.path.insert(0, _p)

import numpy as np
import ml_dtypes

import concourse.bass as bass
import concourse.mybir as mybir
import concourse.tile as tile
from concourse import bacc
from concourse import bass_utils
from concourse.bass import IndirectOffsetOnAxis

BF16 = mybir.dt.bfloat16
F32 = mybir.dt.float32
P = 128
NCORES = 8
GATE_GRP = 4  # chunks per gate batch (512 edges)

LAST_RESULTS = None  # test harness introspection


def _host_prep(x, edge_index, edge_attr, emb_W, emb_b, edge_W, edge_b,
               U_W, U_b, V_W, V_b, A_W, A_b, B_W, B_b, E_W, E_b, ln_g, ln_b):
    N, IN_DIM = x.shape
    E = edge_index.shape[1]
    ED = edge_attr.shape[1]
    H = emb_W.shape[1]
    assert IN_DIM == H == P

    bpc = -(-N // (NCORES * P))          # blocks per core
    nblk = NCORES * bpc                  # total 128-node blocks
    npad = nblk * P
    nloc = bpc * P                       # node slots per core

    src = np.ascontiguousarray(edge_index[0]).astype(np.int64)
    dst = np.ascontiguousarray(edge_index[1]).astype(np.int64)

    # --- degree-balanced node->block assignment (snake deal of sorted degrees)
    deg = np.bincount(dst, minlength=npad)
    order_nodes = np.argsort(-deg, kind="stable")    # high degree first
    rounds = npad // nblk if npad % nblk == 0 else None
    assert npad % nblk == 0
    rounds = npad // nblk                            # = 128
    grid = order_nodes.reshape(rounds, nblk).copy()
    grid[1::2] = grid[1::2, ::-1]                    # snake to cancel bias
    # node grid[r, b] -> new id b*128 + r
    perm = np.empty(npad, dtype=np.int64)
    newids = (np.arange(nblk)[None, :] * P + np.arange(rounds)[:, None])
    perm[grid] = newids
    perm32 = perm.astype(np.int32)

    src_n = perm[src]
    dst_n = perm[dst]

    # --- sort edges by new dst, pad each block's run to a uniform capacity
    eorder = np.argsort(dst_n, kind="stable")
    src_s = src_n[eorder].astype(np.int32)
    dst_s = dst_n[eorder].astype(np.int32)
    ea_s = np.asarray(edge_attr, np.float32)[eorder]

    blk_of_edge = dst_s >> 7
    counts = np.bincount(blk_of_edge, minlength=nblk)
    cap_chunks = int(-(-counts.max() // P))
    cap_chunks = -(-cap_chunks // GATE_GRP) * GATE_GRP   # multiple of gate group
    cap = cap_chunks * P
    epad = nblk * cap

    run_start = np.zeros(nblk, dtype=np.int64)
    run_start[1:] = np.cumsum(counts)[:-1]
    rank = np.arange(E, dtype=np.int64) - run_start[blk_of_edge]
    pos = blk_of_edge.astype(np.int64) * cap + rank

    src_p = np.zeros(epad, dtype=np.int32)
    dst_p = np.zeros(epad, dtype=np.int32)
    dloc_p = np.full(epad, 255, dtype=np.float32)     # 255 => one-hot all-zero
    ea_p = np.zeros((epad, ED), dtype=np.float32)
    src_p[pos] = src_s
    dst_p[pos] = dst_s
    dloc_p[pos] = (dst_s & 127).astype(np.float32)
    ea_p[pos] = ea_s

    # --- fold weights (float64 host math, exact reassociation of reference)
    f8 = lambda a: np.asarray(a, np.float64)
    A2 = f8(emb_W) @ f8(A_W); a2 = f8(emb_b) @ f8(A_W) + f8(A_b)
    U2 = f8(emb_W) @ f8(U_W); u2 = f8(emb_b) @ f8(U_W) + f8(U_b)
    V2 = f8(emb_W) @ f8(V_W); v2 = f8(emb_b) @ f8(V_W) + f8(V_b)
    W2 = f8(edge_W) @ f8(E_W)
    b2 = f8(edge_b) @ f8(E_W) + f8(E_b) + a2 + v2

    bf = lambda a: np.ascontiguousarray(np.asarray(a, np.float32).astype(ml_dtypes.bfloat16))
    f32c = lambda a: np.ascontiguousarray(np.asarray(a, np.float32))

    consts = {
        "w2p": bf(np.concatenate([W2, b2[None, :]], axis=0)),       # [ED+1,128]
        "auvw": bf(np.concatenate([A2, U2, V2], axis=1)),           # [128,384]
        "u2row": bf(u2[None, :]),                                   # [1,128]
        "embw": f32c(emb_W),
        "bw": f32c(B_W),
        "cb": f32c(np.tile((f8(emb_b) + f8(B_b))[None, :], (P, 1))),
        "iota": bf(np.tile(np.arange(P, dtype=np.float32)[None, :], (P, 1))),
        "ident": bf(np.eye(P, dtype=np.float32)),
        "onescol": bf(np.ones((1, P), np.float32)),
    }
    ln_affine = not (np.allclose(np.asarray(ln_g), 1.0) and np.allclose(np.asarray(ln_b), 0.0))
    if ln_affine:
        consts["gb"] = f32c(np.tile(np.asarray(ln_g, np.float32)[None, :], (P, 1)))
        consts["bb"] = f32c(np.tile(np.asarray(ln_b, np.float32)[None, :], (P, 1)))

    # --- x in permuted space
    x_perm = np.zeros((npad, P), dtype=np.float32)
    x_perm[perm32[:N]] = np.asarray(x, np.float32)
    xt_bf = np.ascontiguousarray(x_perm.T.astype(ml_dtypes.bfloat16))   # [128, npad]

    # --- per-core arrays
    ecore = bpc * cap
    ccore = bpc * cap_chunks
    per_core = []
    for c in range(NCORES):
        s, e = c * ecore, (c + 1) * ecore
        eaT = np.concatenate([ea_p[s:e].T, np.ones((1, ecore), np.float32)], axis=0)
        per_core.append({
            "eat": np.ascontiguousarray(eaT.astype(ml_dtypes.bfloat16)),          # [ED+1, ecore]
            "dstloc": np.ascontiguousarray(dloc_p[s:e].reshape(ccore, P).T),       # [128, ccore] f32
            "srcidx": np.ascontiguousarray(src_p[s:e].reshape(ccore, P).T),        # [128, ccore] i32
            "dstidx": np.ascontiguousarray(dst_p[s:e].reshape(ccore, P).T),        # [128, ccore] i32
            "xtl": np.ascontiguousarray(x_perm[c * nloc:(c + 1) * nloc].T),        # [128, nloc] f32
        })

    consts["xt"] = xt_bf
    meta = dict(N=N, E=E, ED=ED, npad=npad, nloc=nloc, bpc=bpc,
                cap_chunks=cap_chunks, cap=cap, ccore=ccore, ecore=ecore,
                perm32=perm32, ln_affine=ln_affine)
    return consts, per_core, meta


def _build_program(nc, tc, meta):
    ED = meta["ED"]
    npad, nloc, bpc = meta["npad"], meta["nloc"], meta["bpc"]
    cap_chunks, ccore = meta["cap_chunks"], meta["ccore"]
    ln_affine = meta["ln_affine"]
    Alu = mybir.AluOpType
    Act = mybir.ActivationFunctionType

    din = {}
    def dram_in(name, shape, dt):
        din[name] = nc.dram_tensor(name, shape, dt, kind="ExternalInput").ap()
        return din[name]

    xt_d = dram_in("xt", [P, npad], BF16)
    xtl_d = dram_in("xtl", [P, nloc], F32)
    eat_d = dram_in("eat", [ED + 1, meta["ecore"]], BF16)
    dstloc_d = dram_in("dstloc", [P, ccore], F32)
    srcidx_d = dram_in("srcidx", [P, ccore], mybir.dt.int32)
    dstidx_d = dram_in("dstidx", [P, ccore], mybir.dt.int32)
    w2p_d = dram_in("w2p", [ED + 1, P], BF16)
    auvw_d = dram_in("auvw", [P, 3 * P], BF16)
    u2_d = dram_in("u2row", [1, P], BF16)
    embw_d = dram_in("embw", [P, P], F32)
    bw_d = dram_in("bw", [P, P], F32)
    cb_d = dram_in("cb", [P, P], F32)
    iota_d = dram_in("iota", [P, P], BF16)
    ident_d = dram_in("ident", [P, P], BF16)
    ones_d = dram_in("onescol", [1, P], BF16)
    if ln_affine:
        gb_d = dram_in("gb", [P, P], F32)
        bb_d = dram_in("bb", [P, P], F32)
    out_d = nc.dram_tensor("out", [nloc, P], F32, kind="ExternalOutput").ap()

    ctx = ExitStack()
    with ctx:
        cpool = ctx.enter_context(tc.tile_pool(name="const", bufs=1))
        dpool = ctx.enter_context(tc.tile_pool(name="dram", bufs=1, space="DRAM"))

        def load_const(src_ap, shape, dt, tag):
            t = cpool.tile(shape, dt, tag=tag)
            nc.sync.dma_start(out=t[:], in_=src_ap[:])
            return t

        w2p_sb = load_const(w2p_d, [ED + 1, P], BF16, "c_w2p")
        auvw_sb = load_const(auvw_d, [P, 3 * P], BF16, "c_auvw")
        u2_sb = load_const(u2_d, [1, P], BF16, "c_u2")
        embw_sb = load_const(embw_d, [P, P], F32, "c_embw")
        bw_sb = load_const(bw_d, [P, P], F32, "c_bw")
        cb_sb = load_const(cb_d, [P, P], F32, "c_cb")
        iota_sb = load_const(iota_d, [P, P], BF16, "c_iota")
        ident_sb = load_const(ident_d, [P, P], BF16, "c_ident")
        ones_sb = load_const(ones_d, [1, P], BF16, "c_ones")
        if ln_affine:
            gb_sb = load_const(gb_d, [P, P], F32, "c_gb")
            bb_sb = load_const(bb_d, [P, P], F32, "c_bb")
        xtl_sb = load_const(xtl_d, [P, nloc], F32, "c_xtl")
        dstloc_sb = load_const(dstloc_d, [P, ccore], F32, "c_dstloc")
        srcidx_sb = load_const(srcidx_d, [P, ccore], mybir.dt.int32, "c_srcidx")
        dstidx_sb = load_const(dstidx_d, [P, ccore], mybir.dt.int32, "c_dstidx")

        au_tab = dpool.tile([npad, 2 * P], BF16)
        vh_tab = dpool.tile([npad, P], BF16)

        # ---------------- Phase A: node tables AU = x@[A2|U2] (+u2 on U), Vh = x@V2
        GB = 512
        with tc.tile_pool(name="pa", bufs=3) as pa, \
             tc.tile_pool(name="pap", bufs=2, space="PSUM") as pap:
            for nb0 in range(0, npad, GB):
                xt_t = pa.tile([P, GB], BF16, tag="xt")
                nc.sync.dma_start(out=xt_t[:], in_=xt_d[:, nb0:nb0 + GB])
                for j in range(GB // P):
                    ps = pap.tile([P, 3 * P], F32, tag="ps")
                    nc.tensor.matmul(ps[:], lhsT=xt_t[:, j * P:(j + 1) * P],
                                     rhs=auvw_sb[:], start=True, stop=False)
                    nc.tensor.matmul(ps[:, P:2 * P], lhsT=ones_sb[:], rhs=u2_sb[:],
                                     start=False, stop=True, skip_group_check=True)
                    tab_t = pa.tile([P, 3 * P], BF16, tag="tab")
                    if j % 2 == 0:
                        nc.vector.tensor_copy(out=tab_t[:], in_=ps[:])
                    else:
                        nc.scalar.activation(out=tab_t[:], in_=ps[:], func=Act.Copy)
                    r0 = nb0 + j * P
                    nc.sync.dma_start(out=au_tab[r0:r0 + P, :], in_=tab_t[:, 0:2 * P])
                    nc.sync.dma_start(out=vh_tab[r0:r0 + P, :], in_=tab_t[:, 2 * P:3 * P])

        # ---------------- Phase B: edge pipeline + per-block residual/LN
        ngrp = cap_chunks // GATE_GRP
        with tc.tile_pool(name="pb", bufs=4) as pb, \
             tc.tile_pool(name="pbg", bufs=3) as pbg, \
             tc.tile_pool(name="p0p", bufs=2, space="PSUM") as p0p, \
             tc.tile_pool(name="p1p", bufs=2, space="PSUM") as p1p, \
             tc.tile_pool(name="p2p", bufs=2, space="PSUM") as p2p:
            for blk in range(bpc):
                eat_t = pb.tile([ED + 1, meta["cap"]], BF16, tag="eat")
                nc.sync.dma_start(out=eat_t[:],
                                  in_=eat_d[:, blk * meta["cap"]:(blk + 1) * meta["cap"]])
                p1 = p1p.tile([P, P], F32, tag="p1")
                for g in range(ngrp):
                    au4 = pbg.tile([P, GATE_GRP * 2 * P], BF16, tag="au4")
                    s4 = pbg.tile([P, GATE_GRP * P], BF16, tag="s4")
                    p0 = p0p.tile([P, GATE_GRP * P], F32, tag="p0")
                    for j in range(GATE_GRP):
                        c = blk * cap_chunks + g * GATE_GRP + j
                        nc.gpsimd.indirect_dma_start(
                            out=au4[:, j * 2 * P:(j + 1) * 2 * P], out_offset=None,
                            in_=au_tab[:, :],
                            in_offset=IndirectOffsetOnAxis(ap=srcidx_sb[:, c:c + 1], axis=0))
                        # Vh[dst] accumulated onto the Ah half -> one identity-matmul adds both
                        nc.gpsimd.indirect_dma_start(
                            out=au4[:, j * 2 * P:j * 2 * P + P], out_offset=None,
                            in_=vh_tab[:, :],
                            in_offset=IndirectOffsetOnAxis(ap=dstidx_sb[:, c:c + 1], axis=0),
                            compute_op=Alu.add)
                    for j in range(GATE_GRP):
                        c = blk * cap_chunks + g * GATE_GRP + j
                        ec = (g * GATE_GRP + j) * P
                        js = slice(j * P, (j + 1) * P)
                        nc.tensor.matmul(p0[:, js], lhsT=eat_t[:, ec:ec + P],
                                         rhs=w2p_sb[:], start=True, stop=False)
                        nc.tensor.matmul(p0[:, js], lhsT=ident_sb[:],
                                         rhs=au4[:, j * 2 * P:j * 2 * P + P],
                                         start=False, stop=True)
                        nc.vector.tensor_scalar(out=s4[:, js], in0=iota_sb[:],
                                                scalar1=dstloc_sb[:, c:c + 1],
                                                scalar2=None, op0=Alu.is_equal)
                    gate4 = pbg.tile([P, GATE_GRP * P], BF16, tag="gate")
                    nc.scalar.activation(out=gate4[:], in_=p0[:], func=Act.Sigmoid)
                    msg4 = pbg.tile([P, GATE_GRP * P], BF16, tag="msg")
                    uh_ap = au4[:].rearrange("p (c e) -> p c e", e=2 * P)[:, :, P:2 * P]
                    nc.vector.tensor_tensor(
                        out=msg4[:].rearrange("p (c e) -> p c e", e=P),
                        in0=gate4[:].rearrange("p (c e) -> p c e", e=P),
                        in1=uh_ap, op=Alu.mult)
                    for j in range(GATE_GRP):
                        js = slice(j * P, (j + 1) * P)
                        nc.tensor.matmul(p1[:], lhsT=msg4[:, js], rhs=s4[:, js],
                                         start=(g == 0 and j == 0),
                                         stop=(g == ngrp - 1 and j == GATE_GRP - 1))
                # ---- block tail: out = LN(h + aggr@B_W + c)
                aggT = pb.tile([P, P], F32, tag="aggT")
                nc.vector.tensor_copy(out=aggT[:], in_=p1[:])
                p2 = p2p.tile([P, P], F32, tag="p2")
                nc.tensor.matmul(p2[:], lhsT=aggT[:], rhs=bw_sb[:], start=True, stop=False)
                nc.tensor.matmul(p2[:], lhsT=xtl_sb[:, blk * P:(blk + 1) * P],
                                 rhs=embw_sb[:], start=False, stop=True)
                v = pb.tile([P, P], F32, tag="v")
                nc.vector.tensor_tensor(out=v[:], in0=p2[:], in1=cb_sb[:], op=Alu.add)
                sum_t = pb.tile([P, 1], F32, tag="sum")
                nc.vector.tensor_reduce(out=sum_t[:], in_=v[:],
                                        axis=mybir.AxisListType.X, op=Alu.add)
                mu_t = pb.tile([P, 1], F32, tag="mu")
                nc.vector.tensor_scalar(out=mu_t[:], in0=sum_t[:], scalar1=1.0 / P,
                                        scalar2=None, op0=Alu.mult)
                vc = pb.tile([P, P], F32, tag="vc")
                nc.vector.tensor_scalar(out=vc[:], in0=v[:], scalar1=mu_t[:, :1],
                                        scalar2=None, op0=Alu.subtract)
                sq = pb.tile([P, P], F32, tag="sq")
                nc.vector.tensor_tensor(out=sq[:], in0=vc[:], in1=vc[:], op=Alu.mult)
                var_t = pb.tile([P, 1], F32, tag="var")
                nc.vector.tensor_reduce(out=var_t[:], in_=sq[:],
                                        axis=mybir.AxisListType.X, op=Alu.add)
                nc.vector.tensor_scalar(out=var_t[:], in0=var_t[:], scalar1=1.0 / P,
                                        scalar2=1e-5, op0=Alu.mult, op1=Alu.add)
                rvar = pb.tile([P, 1], F32, tag="rvar")
                nc.vector.reciprocal(out=rvar[:], in_=var_t[:])
                rstd = pb.tile([P, 1], F32, tag="rstd")
                nc.scalar.activation(out=rstd[:], in_=rvar[:], func=Act.Sqrt)
                outb = pb.tile([P, P], F32, tag="outb")
                nc.vector.tensor_scalar(out=outb[:], in0=vc[:], scalar1=rstd[:, :1],
                                        scalar2=None, op0=Alu.mult)
                if ln_affine:
                    nc.vector.tensor_tensor(out=outb[:], in0=outb[:], in1=gb_sb[:], op=Alu.mult)
                    nc.vector.tensor_tensor(out=outb[:], in0=outb[:], in1=bb_sb[:], op=Alu.add)
                nc.sync.dma_start(out=out_d[blk * P:(blk + 1) * P, :], in_=outb[:])


def _build(inputs):
    consts, per_core, meta = _host_prep(**inputs)
    nc = bacc.Bacc("TRN2", target_bir_lowering=False, debug=False,
                   num_devices=NCORES)
    with tile.TileContext(nc) as tc:
        _build_program(nc, tc, meta)
    nc.compile()
    in_maps = [{**consts, **per_core[c]} for c in range(NCORES)]
    return dict(nc=nc, in_maps=in_maps, meta=meta)


def _exec(ctx):
    global LAST_RESULTS
    res = bass_utils.run_bass_kernel_spmd(
        ctx["nc"], ctx["in_maps"], core_ids=list(range(NCORES)), trace=False)
    LAST_RESULTS = res
    meta = ctx["meta"]
    big = np.concatenate([res.results[c]["out"] for c in range(NCORES)], axis=0)
    out = big[meta["perm32"][:meta["N"]]]
    return np.ascontiguousarray(out, dtype=np.float32)


def _timeit(ctx, iters=5):
    """Steady-state per-call wall time with device-resident inputs (upper
    bound on HW exec: includes dispatch/axon overhead but no H2D)."""
    import time
    import jax
    from jax.experimental.shard_map import shard_map
    from jax.sharding import Mesh, PartitionSpec, NamedSharding
    from concourse import bass2jax as b2j
    from concourse import mybir as _mb

    nc = ctx["nc"]
    in_maps = ctx["in_maps"]
    in_names, out_names, out_avals, zero_outs = [], [], [], []
    part_name = nc.partition_id_tensor.name if nc.partition_id_tensor else None
    for alloc in nc.m.functions[0].allocations:
        if not isinstance(alloc, _mb.MemoryLocationSet):
            continue
        name = alloc.memorylocations[0].name
        if alloc.kind == "ExternalInput":
            if name != part_name:
                in_names.append(name)
        elif alloc.kind == "ExternalOutput":
            out_names.append(name)
            shape = tuple(alloc.tensor_shape)
            dtype = _mb.dt.np(alloc.dtype)
            out_avals.append(jax.core.ShapedArray(shape, dtype))
            zero_outs.append(np.zeros(shape, dtype))
    n_params = len(in_names)
    all_names = in_names + out_names
    if part_name is not None:
        all_names = all_names + [part_name]

    def _body(*args):
        operands = list(args)
        if part_name is not None:
            operands.append(b2j.partition_id_tensor())
        outs = b2j._bass_exec_p.bind(
            *operands, out_avals=tuple(out_avals), in_names=tuple(all_names),
            out_names=tuple(out_names), lowering_input_output_aliases=(),
            sim_require_finite=True, sim_require_nnan=True, nc=nc)
        return tuple(outs)

    devices = jax.devices()[:NCORES]
    mesh = Mesh(np.asarray(devices), ("core",))
    spec = PartitionSpec("core")
    n_outs = len(out_names)
    fn = jax.jit(shard_map(_body, mesh=mesh,
                           in_specs=(spec,) * (n_params + n_outs),
                           out_specs=(spec,) * n_outs, check_rep=False))
    sharding = NamedSharding(mesh, spec)
    dev_in = [jax.device_put(
        np.concatenate([np.asarray(in_maps[c][nm]) for c in range(NCORES)], axis=0),
        sharding) for nm in in_names]
    dev_zero = [jax.device_put(
        np.zeros((NCORES * z.shape[0], *z.shape[1:]), z.dtype), sharding)
        for z in zero_outs]
    times = []
    out = None
    for _ in range(iters):
        t0 = time.perf_counter()
        out = fn(*dev_in, *dev_zero)
        jax.block_until_ready(out)
        times.append(time.perf_counter() - t0)
    return times, out


def kernel(**inputs) -> np.ndarray:
    return _exec(_build(inputs))



# revision 8
# speedup vs baseline: 1.2054x; 1.2054x over previous
"""GatedGCN LocalEncoder kernel for 8x Trainium2 NeuronCores (Bass/Tile).

Strategy: destination-sorted edge sharding with dma_gather-based source
gathers. Nodes are relabeled into degree-balanced 128-node blocks (784
blocks, 98 per core). All edges with dst in a block form one contiguous
run, sub-sorted by source quarter (4 tables of 25088 rows so gather
indices fit int16). Per block:
  - 4 transposed dma_gathers fetch x[src]^T (bf16) from the quarter tables
  - 1 SBUF-source dma_gather fetches onehot(dst_local) columns from a
    resident identity tile (used to broadcast the per-block Vh and the u2
    bias into the gate PSUM with a single matmul)
  - per 128-edge chunk: 3 gate matmuls (A|U projection from gathered x^T,
    edge-feature term, Vh/u2 one-hot term), sigmoid, msg multiply, and a
    one-hot segment-sum matmul accumulated in PSUM
  - block tail: residual + LayerNorm as two f32 matmuls + DVE ops.
No cross-core communication; each core owns 98 blocks of destinations.
"""

import os
import sys
from contextlib import ExitStack

for _p in ("/opt/trn_rl_repo", os.path.expanduser("~/.axon_site/_ro/trn_rl_repo")):
    if os.path.isdir(_p) and _p not in sys.path:
        sys.path.insert(0, _p)

import numpy as np
import ml_dtypes

import concourse.bass as bass
import concourse.mybir as mybir
import concourse.tile as tile
from concourse import bacc
from concourse import bass_utils
from concourse import library_config

BF16 = mybir.dt.bfloat16
F32 = mybir.dt.float32
I16 = mybir.dt.int16
P = 128
NCORES = 8
NQ = 4  # source-quarter tables (int16 gather index limit)

LAST_RESULTS = None  # test harness introspection


def _wrap_idx16(vals, runs, run_len):
    """vals [runs*run_len] int -> [128, runs*run_len//16] int16 in dma_gather's
    wrapped layout (idx i of a run -> partition i%16, col i//16; replicated
    across the 8 groups of 16 partitions)."""
    arr = np.asarray(vals, np.int16).reshape(runs, run_len // 16, 16)
    arr = np.ascontiguousarray(arr.transpose(2, 0, 1).reshape(16, -1))
    return np.ascontiguousarray(np.tile(arr, (8, 1)))


def _host_prep(x, edge_index, edge_attr, emb_W, emb_b, edge_W, edge_b,
               U_W, U_b, V_W, V_b, A_W, A_b, B_W, B_b, E_W, E_b, ln_g, ln_b):
    N, IN_DIM = x.shape
    E = edge_index.shape[1]
    ED = edge_attr.shape[1]
    H = emb_W.shape[1]
    assert IN_DIM == H == P

    bpc = -(-N // (NCORES * P))          # blocks per core
    nblk = NCORES * bpc                  # total 128-node blocks
    npad = nblk * P
    nloc = bpc * P                       # node slots per core
    assert npad % NQ == 0
    qrows = npad // NQ

    src = np.ascontiguousarray(edge_index[0]).astype(np.int64)
    dst = np.ascontiguousarray(edge_index[1]).astype(np.int64)

    # --- degree-balanced node->block assignment (snake deal of sorted degrees)
    deg = np.bincount(dst, minlength=npad)
    order_nodes = np.argsort(-deg, kind="stable")
    assert npad % nblk == 0
    rounds = npad // nblk
    grid = order_nodes.reshape(rounds, nblk).copy()
    grid[1::2] = grid[1::2, ::-1]
    perm = np.empty(npad, dtype=np.int64)
    newids = (np.arange(nblk)[None, :] * P + np.arange(rounds)[:, None])
    perm[grid] = newids
    perm32 = perm.astype(np.int32)

    src_n = perm[src]
    dst_n = perm[dst]

    # --- sort edges by (dst block, src quarter); pad each run to capq
    q_of = src_n // qrows
    blk_of = dst_n >> 7
    key = blk_of * NQ + q_of
    eorder = np.argsort(key, kind="stable")
    key_s = key[eorder]
    src_s = src_n[eorder]
    dst_s = dst_n[eorder]
    q_s = q_of[eorder]
    ea_s = np.asarray(edge_attr, np.float32)[eorder]

    counts = np.bincount(key_s, minlength=nblk * NQ)
    capq = int(-(-counts.max() // P)) * P
    capb = NQ * capq                      # edge capacity per block
    nch = capb // P                       # chunks per block
    ccore = bpc * nch
    ecore = bpc * capb
    epad = nblk * capb

    run_start = np.zeros(nblk * NQ, dtype=np.int64)
    run_start[1:] = np.cumsum(counts)[:-1]
    rank = np.arange(E, dtype=np.int64) - run_start[key_s]
    pos = key_s * capq + rank

    srcq_p = np.zeros(epad, dtype=np.int16)      # pad 0 -> gathers row 0 (benign)
    dloc16_p = np.zeros(epad, dtype=np.int16)    # pad 0 -> onehot(0) (benign)
    dlocf_p = np.full(epad, 255, dtype=np.float32)  # pad 255 -> s4 all-zero row
    ea_p = np.zeros((epad, ED), dtype=np.float32)
    srcq_p[pos] = (src_s - q_s * qrows).astype(np.int16)
    dloc16_p[pos] = (dst_s & 127).astype(np.int16)
    dlocf_p[pos] = (dst_s & 127).astype(np.float32)
    ea_p[pos] = ea_s

    # --- fold weights (float64 host math, exact reassociation of reference)
    f8 = lambda a: np.asarray(a, np.float64)
    A2 = f8(emb_W) @ f8(A_W)
    U2 = f8(emb_W) @ f8(U_W)
    u2 = f8(emb_b) @ f8(U_W) + f8(U_b)
    V2 = f8(emb_W) @ f8(V_W)
    a2 = f8(emb_b) @ f8(A_W) + f8(A_b)
    v2 = f8(emb_b) @ f8(V_W) + f8(V_b)
    W2 = f8(edge_W) @ f8(E_W)
    b2 = f8(edge_b) @ f8(E_W) + f8(E_b) + a2 + v2

    bf = lambda a: np.ascontiguousarray(np.asarray(a, np.float32).astype(ml_dtypes.bfloat16))
    f32c = lambda a: np.ascontiguousarray(np.asarray(a, np.float32))

    consts = {
        "w2p": bf(np.concatenate([W2, b2[None, :]], axis=0)),       # [ED+1,128]
        "au2": bf(np.concatenate([A2, U2], axis=1)),                # [128,256]
        "v2w": f32c(V2),                                            # [128,128]
        "u2b": bf(np.tile(np.asarray(u2, np.float32)[None, :], (P, 1))),
        "embw": f32c(emb_W),
        "bw": f32c(B_W),
        "cb": f32c(np.tile((f8(emb_b) + f8(B_b))[None, :], (P, 1))),
        "iota": bf(np.tile(np.arange(P, dtype=np.float32)[None, :], (P, 1))),
        "identoh": bf(np.eye(P, dtype=np.float32)),
    }
    ln_affine = not (np.allclose(np.asarray(ln_g), 1.0) and np.allclose(np.asarray(ln_b), 0.0))
    if ln_affine:
        consts["gb"] = f32c(np.tile(np.asarray(ln_g, np.float32)[None, :], (P, 1)))
        consts["bb"] = f32c(np.tile(np.asarray(ln_b, np.float32)[None, :], (P, 1)))

    # --- x in permuted space: quarter tables (bf16 rows) + per-core f32 cols
    x_perm = np.zeros((npad, P), dtype=np.float32)
    x_perm[perm32[:N]] = np.asarray(x, np.float32)
    x_bf = np.ascontiguousarray(x_perm.astype(ml_dtypes.bfloat16))
    for q in range(NQ):
        consts[f"xq{q}"] = np.ascontiguousarray(x_bf[q * qrows:(q + 1) * qrows])

    # --- per-core arrays
    per_core = []
    for c in range(NCORES):
        s, e = c * ecore, (c + 1) * ecore
        eaT = np.concatenate([ea_p[s:e].T, np.ones((1, ecore), np.float32)], axis=0)
        per_core.append({
            "eat": np.ascontiguousarray(eaT.astype(ml_dtypes.bfloat16)),      # [ED+1, ecore]
            "dstloc": np.ascontiguousarray(dlocf_p[s:e].reshape(ccore, P).T),  # [128, ccore] f32
            "sidx": _wrap_idx16(srcq_p[s:e], bpc * NQ, capq),                  # [128, bpc*NQ*capq/16]
            "didx": _wrap_idx16(dloc16_p[s:e], bpc, capb),                     # [128, bpc*capb/16]
            "xtl": np.ascontiguousarray(x_perm[c * nloc:(c + 1) * nloc].T),    # [128, nloc] f32
        })

    meta = dict(N=N, E=E, ED=ED, npad=npad, nloc=nloc, bpc=bpc, qrows=qrows,
                capq=capq, capb=capb, nch=nch, ccore=ccore, ecore=ecore,
                perm32=perm32, ln_affine=ln_affine)
    return consts, per_core, meta


def _build_program(nc, tc, meta):
    ED = meta["ED"]
    nloc, bpc = meta["nloc"], meta["bpc"]
    qrows, capq, capb, nch, ccore = (
        meta["qrows"], meta["capq"], meta["capb"], meta["nch"], meta["ccore"])
    ln_affine = meta["ln_affine"]
    Alu = mybir.AluOpType
    Act = mybir.ActivationFunctionType
    cq16 = capq // 16
    cb16 = capb // 16

    def dram_in(name, shape, dt):
        return nc.dram_tensor(name, shape, dt, kind="ExternalInput").ap()

    xq_d = [dram_in(f"xq{q}", [qrows, P], BF16) for q in range(NQ)]
    w2p_d = dram_in("w2p", [ED + 1, P], BF16)
    au2_d = dram_in("au2", [P, 2 * P], BF16)
    v2_d = dram_in("v2w", [P, P], F32)
    u2b_d = dram_in("u2b", [P, P], BF16)
    embw_d = dram_in("embw", [P, P], F32)
    bw_d = dram_in("bw", [P, P], F32)
    cb_d = dram_in("cb", [P, P], F32)
    iota_d = dram_in("iota", [P, P], BF16)
    identoh_d = dram_in("identoh", [P, P], BF16)
    if ln_affine:
        gb_d = dram_in("gb", [P, P], F32)
        bb_d = dram_in("bb", [P, P], F32)
    eat_d = dram_in("eat", [ED + 1, meta["ecore"]], BF16)
    dstloc_d = dram_in("dstloc", [P, ccore], F32)
    sidx_d = dram_in("sidx", [P, bpc * NQ * cq16], I16)
    didx_d = dram_in("didx", [P, bpc * cb16], I16)
    xtl_d = dram_in("xtl", [P, nloc], F32)
    out_d = nc.dram_tensor("out", [nloc, P], F32, kind="ExternalOutput").ap()

    nc.gpsimd.load_library(library_config.mlp)

    ctx = ExitStack()
    with ctx:
        cpool = ctx.enter_context(tc.tile_pool(name="const", bufs=1))

        def load_const(src_ap, shape, dt, tag):
            t = cpool.tile(shape, dt, tag=tag)
            nc.sync.dma_start(out=t[:], in_=src_ap[:])
            return t

        w2p_sb = load_const(w2p_d, [ED + 1, P], BF16, "c_w2p")
        au2_sb = load_const(au2_d, [P, 2 * P], BF16, "c_au2")
        v2_sb = load_const(v2_d, [P, P], F32, "c_v2")
        u2b_sb = load_const(u2b_d, [P, P], BF16, "c_u2b")
        embw_sb = load_const(embw_d, [P, P], F32, "c_embw")
        bw_sb = load_const(bw_d, [P, P], F32, "c_bw")
        cb_sb = load_const(cb_d, [P, P], F32, "c_cb")
        iota_sb = load_const(iota_d, [P, P], BF16, "c_iota")
        identoh_sb = load_const(identoh_d, [P, P], BF16, "c_identoh")
        if ln_affine:
            gb_sb = load_const(gb_d, [P, P], F32, "c_gb")
            bb_sb = load_const(bb_d, [P, P], F32, "c_bb")
        dstloc_sb = load_const(dstloc_d, [P, ccore], F32, "c_dstloc")

        with tc.tile_pool(name="pb", bufs=3) as pb, \
             tc.tile_pool(name="pc", bufs=4) as pc, \
             tc.tile_pool(name="paup", bufs=4, space="PSUM") as paup, \
             tc.tile_pool(name="p1p", bufs=2, space="PSUM") as p1p:
            for blk in range(bpc):
                # ---- block loads
                eat_t = pb.tile([ED + 1, capb], BF16, tag="eat")
                nc.sync.dma_start(out=eat_t[:],
                                  in_=eat_d[:, blk * capb:(blk + 1) * capb])
                sidx_t = pb.tile([P, NQ * cq16], I16, tag="sidx")
                nc.sync.dma_start(out=sidx_t[:],
                                  in_=sidx_d[:, blk * NQ * cq16:(blk + 1) * NQ * cq16])
                didx_t = pb.tile([P, cb16], I16, tag="didx")
                nc.sync.dma_start(out=didx_t[:],
                                  in_=didx_d[:, blk * cb16:(blk + 1) * cb16])
                xtl_t = pb.tile([P, P], F32, tag="xtl")
                nc.sync.dma_start(out=xtl_t[:], in_=xtl_d[:, blk * P:(blk + 1) * P])

                # ---- gathers: x[src]^T per quarter + onehot(dst) columns
                xg_t = pb.tile([P, capb], BF16, tag="xg")
                for q in range(NQ):
                    nc.gpsimd.dma_gather(
                        out_ap=xg_t[:, q * capq:(q + 1) * capq]
                            .rearrange("p (o e) -> p o e", o=1),
                        in_ap=xq_d[q][:],
                        idxs_ap=sidx_t[:, q * cq16:(q + 1) * cq16],
                        num_idxs=capq,
                        num_idxs_reg=capq,
                        elem_size=P,
                        transpose=True,
                        single_packet=False,
                    )
                oh_t = pb.tile([P, capb], BF16, tag="oh")
                nc.gpsimd.dma_gather(
                    out_ap=oh_t[:].rearrange("p (o e) -> p o e", o=1),
                    in_ap=identoh_sb[:],
                    idxs_ap=didx_t[:],
                    num_idxs=capb,
                    num_idxs_reg=capb,
                    elem_size=P,
                    transpose=True,
                    single_packet=False,
                    sbuf_tokens_per_rank=P,
                    sbuf_free_dim_per_rank=2 * P,
                )

                # ---- per-block Vh table (f32 matmul) + u2 bias half
                vhb_ps = paup.tile([P, 2 * P], F32, tag="pau")
                nc.tensor.matmul(vhb_ps[:, 0:P], lhsT=xtl_t[:], rhs=v2_sb[:],
                                 start=True, stop=True)
                vhbu2_t = pb.tile([P, 2 * P], BF16, tag="vhbu2")
                nc.vector.tensor_copy(out=vhbu2_t[:, 0:P], in_=vhb_ps[:, 0:P])
                nc.vector.tensor_copy(out=vhbu2_t[:, P:2 * P], in_=u2b_sb[:])

                # ---- edge chunks
                p1 = p1p.tile([P, P], F32, tag="p1")
                for c in range(nch):
                    cs = slice(c * P, (c + 1) * P)
                    pau = paup.tile([P, 2 * P], F32, tag="pau")
                    nc.tensor.matmul(pau[:], lhsT=xg_t[:, cs], rhs=au2_sb[:],
                                     start=True, stop=False)
                    nc.tensor.matmul(pau[:, 0:P], lhsT=eat_t[:, cs], rhs=w2p_sb[:],
                                     start=False, stop=False, skip_group_check=True)
                    nc.tensor.matmul(pau[:], lhsT=oh_t[:, cs], rhs=vhbu2_t[:],
                                     start=False, stop=True)
                    gate_t = pc.tile([P, P], BF16, tag="gate")
                    nc.scalar.activation(out=gate_t[:], in_=pau[:, 0:P],
                                         func=Act.Sigmoid)
                    s4_t = pc.tile([P, P], BF16, tag="s4")
                    gc = blk * nch + c
                    nc.vector.tensor_scalar(out=s4_t[:], in0=iota_sb[:],
                                            scalar1=dstloc_sb[:, gc:gc + 1],
                                            scalar2=None, op0=Alu.is_equal)
                    msg_t = pc.tile([P, P], BF16, tag="msg")
                    nc.vector.tensor_tensor(out=msg_t[:], in0=gate_t[:],
                                            in1=pau[:, P:2 * P], op=Alu.mult)
                    nc.tensor.matmul(p1[:], lhsT=msg_t[:], rhs=s4_t[:],
                                     start=(c == 0), stop=(c == nch - 1))

                # ---- block tail: out = LN(h + aggr@B_W + cb)
                aggT = pb.tile([P, P], F32, tag="aggT")
                nc.vector.tensor_copy(out=aggT[:], in_=p1[:])
                p2 = paup.tile([P, 2 * P], F32, tag="pau")
                nc.tensor.matmul(p2[:, 0:P], lhsT=aggT[:], rhs=bw_sb[:], start=True, stop=False)
                nc.tensor.matmul(p2[:, 0:P], lhsT=xtl_t[:], rhs=embw_sb[:],
                                 start=False, stop=True)
                v = pb.tile([P, P], F32, tag="v")
                nc.vector.tensor_tensor(out=v[:], in0=p2[:, 0:P], in1=cb_sb[:], op=Alu.add)
                sum_t = pb.tile([P, 1], F32, tag="sum")
                nc.vector.tensor_reduce(out=sum_t[:], in_=v[:],
                                        axis=mybir.AxisListType.X, op=Alu.add)
                mu_t = pb.tile([P, 1], F32, tag="mu")
                nc.vector.tensor_scalar(out=mu_t[:], in0=sum_t[:], scalar1=1.0 / P,
                                        scalar2=None, op0=Alu.mult)
                vc = pb.tile([P, P], F32, tag="vc")
                nc.vector.tensor_scalar(out=vc[:], in0=v[:], scalar1=mu_t[:, :1],
                                        scalar2=None, op0=Alu.subtract)
                sq = pb.tile([P, P], F32, tag="sq")
                nc.vector.tensor_tensor(out=sq[:], in0=vc[:], in1=vc[:], op=Alu.mult)
                var_t = pb.tile([P, 1], F32, tag="var")
                nc.vector.tensor_reduce(out=var_t[:], in_=sq[:],
                                        axis=mybir.AxisListType.X, op=Alu.add)
                nc.vector.tensor_scalar(out=var_t[:], in0=var_t[:], scalar1=1.0 / P,
                                        scalar2=1e-5, op0=Alu.mult, op1=Alu.add)
                rvar = pb.tile([P, 1], F32, tag="rvar")
                nc.vector.reciprocal(out=rvar[:], in_=var_t[:])
                rstd = pb.tile([P, 1], F32, tag="rstd")
                nc.scalar.activation(out=rstd[:], in_=rvar[:], func=Act.Sqrt)
                outb = pb.tile([P, P], F32, tag="outb")
                nc.vector.tensor_scalar(out=outb[:], in0=vc[:], scalar1=rstd[:, :1],
                                        scalar2=None, op0=Alu.mult)
                if ln_affine:
                    nc.vector.tensor_tensor(out=outb[:], in0=outb[:], in1=gb_sb[:], op=Alu.mult)
                    nc.vector.tensor_tensor(out=outb[:], in0=outb[:], in1=bb_sb[:], op=Alu.add)
                nc.sync.dma_start(out=out_d[blk * P:(blk + 1) * P, :], in_=outb[:])


def _build(inputs):
    consts, per_core, meta = _host_prep(**inputs)
    nc = bacc.Bacc("TRN2", target_bir_lowering=False, debug=False,
                   num_devices=NCORES)
    with tile.TileContext(nc) as tc:
        _build_program(nc, tc, meta)
    nc.compile()
    in_maps = [{**consts, **per_core[c]} for c in range(NCORES)]
    return dict(nc=nc, in_maps=in_maps, meta=meta)


def _exec(ctx, trace=False):
    global LAST_RESULTS
    res = bass_utils.run_bass_kernel_spmd(
        ctx["nc"], ctx["in_maps"], core_ids=list(range(NCORES)), trace=trace)
    LAST_RESULTS = res
    meta = ctx["meta"]
    big = np.concatenate([res.results[c]["out"] for c in range(NCORES)], axis=0)
    out = big[meta["perm32"][:meta["N"]]]
    return np.ascontiguousarray(out, dtype=np.float32)


def _timeit(ctx, iters=5):
    """Steady-state per-call wall time with device-resident inputs (upper
    bound on HW exec: includes dispatch/axon overhead but no H2D)."""
    import time
    import jax
    from jax.experimental.shard_map import shard_map
    from jax.sharding import Mesh, PartitionSpec, NamedSharding
    from concourse import bass2jax as b2j
    from concourse import mybir as _mb

    nc = ctx["nc"]
    in_maps = ctx["in_maps"]
    in_names, out_names, out_avals, zero_outs = [], [], [], []
    part_name = nc.partition_id_tensor.name if nc.partition_id_tensor else None
    for alloc in nc.m.functions[0].allocations:
        if not isinstance(alloc, _mb.MemoryLocationSet):
            continue
        name = alloc.memorylocations[0].name
        if alloc.kind == "ExternalInput":
            if name != part_name:
                in_names.append(name)
        elif alloc.kind == "ExternalOutput":
            out_names.append(name)
            shape = tuple(alloc.tensor_shape)
            dtype = _mb.dt.np(alloc.dtype)
            out_avals.append(jax.core.ShapedArray(shape, dtype))
            zero_outs.append(np.zeros(shape, dtype))
    n_params = len(in_names)
    all_names = in_names + out_names
    if part_name is not None:
        all_names = all_names + [part_name]

    def _body(*args):
        operands = list(args)
        if part_name is not None:
            operands.append(b2j.partition_id_tensor())
        outs = b2j._bass_exec_p.bind(
            *operands, out_avals=tuple(out_avals), in_names=tuple(all_names),
            out_names=tuple(out_names), lowering_input_output_aliases=(),
            sim_require_finite=True, sim_require_nnan=True, nc=nc)
        return tuple(outs)

    devices = jax.devices()[:NCORES]
    mesh = Mesh(np.asarray(devices), ("core",))
    spec = PartitionSpec("core")
    n_outs = len(out_names)
    fn = jax.jit(shard_map(_body, mesh=mesh,
                           in_specs=(spec,) * (n_params + n_outs),
                           out_specs=(spec,) * n_outs, check_rep=False))
    sharding = NamedSharding(mesh, spec)
    dev_in = [jax.device_put(
        np.concatenate([np.asarray(in_maps[c][nm]) for c in range(NCORES)], axis=0),
        sharding) for nm in in_names]
    dev_zero = [jax.device_put(
        np.zeros((NCORES * z.shape[0], *z.shape[1:]), z.dtype), sharding)
        for z in zero_outs]
    times = []
    out = None
    for _ in range(iters):
        t0 = time.perf_counter()
        out = fn(*dev_in, *dev_zero)
        jax.block_until_ready(out)
        times.append(time.perf_counter() - t0)
    return times, out


def kernel(**inputs) -> np.ndarray:
    return _exec(_build(inputs))
